# revision 2
# baseline (speedup 1.0000x reference)
"""Local2d (unshared-weight conv) Bass kernel for 8 trn2 NeuronCores.

Problem: input (64,64,32,32), weight (32,32,128,64,3,3), bias (128,32,32)
-> out (64,128,32,32).  K=3, stride 1, pad 1.

Sharding: spatial over h_out — core i handles output rows 4i..4i+3 and
reads the disjoint weight slice for those rows, plus a 6-row input halo
slab.

The kernel is DMA-bound (all traffic serializes through the shared DMA
engines at ~360B/ns), so everything on the wire is fp16: weights, input
slab and the output (PSUM accumulation stays fp32; the host casts the
output back to fp32).  Tolerance is 2e-2; fp16 matmul error is ~1e-3.

Per output location (ho,wo) the contraction is over (c,ki,kj) = 576,
packed as 6 PE matmuls accumulating in PSUM:
  - 3 "paired" matmuls, K=128: partitions 0-63 = channels at ki=0,
    partitions 64-127 = channels at ki=1 (the SBUF input slab is loaded
    twice, the upper 64 partitions shifted by one input row so a single
    access-pattern offset addresses both ki rows).
  - 3 "single" matmuls, K=64: channels at ki=2.
Stationary operand = per-location weights [K,128(o)], moving = input
columns [K,64(b)].  Host pre-transposes the weights so the contraction
dim lands on partitions with fully contiguous DMA.
"""

import numpy as np

B, C, O, KK, H, W = 64, 64, 128, 3, 32, 32
HO = WO = 32
NCORES = 8
RPC = HO // NCORES          # output rows per core
LOCS = RPC * WO             # locations per core
G = 8                       # locations per weight-DMA group
NG = LOCS // G


def _build_bass(mode="full", ngroups=None, mix=0, repeat=1):
    from concourse import bacc
    import concourse.mybir as mybir
    from concourse.tile import TileContext

    f16 = mybir.dt.float16
    f32 = mybir.dt.float32
    nc = bacc.Bacc("TRN2", target_bir_lowering=False, debug=False,
                   num_devices=NCORES)

    # exact SBUF image of the input slab: partition-major [128, 6, 34, 64]
    # with zero pads and the h-shifted upper-half copy baked in on host,
    # so the load is a single fully-contiguous DMA.
    slab_d = nc.dram_tensor("slab", (128, RPC + 2, W + 2, B), f16,
                            kind="ExternalInput").ap()
    # weights pre-arranged on host: per group, partition-major, so the
    # DMA is a single fully-contiguous [128, G*3*O] block (6KB runs).
    wp_d = nc.dram_tensor("wp", (NG, 128, G * 3 * O), f16,
                          kind="ExternalInput").ap()
    ws_d = nc.dram_tensor("ws", (NG, 64, G * 3 * O), f16,
                          kind="ExternalInput").ap()
    bias_d = nc.dram_tensor("bias", (O, LOCS), f32,
                            kind="ExternalInput").ap()
    out_d = nc.dram_tensor("out", (RPC, O, WO, B), f16,
                           kind="ExternalOutput").ap()

    with TileContext(nc) as tc:
        with tc.tile_pool(name="xslab", bufs=1) as xpool, \
             tc.tile_pool(name="wpool", bufs=4) as wpool, \
             tc.tile_pool(name="spool", bufs=4) as spool, \
             tc.tile_pool(name="bpool", bufs=1) as bpool, \
             tc.tile_pool(name="opool", bufs=2) as opool, \
             tc.tile_pool(name="psum", bufs=4, space="PSUM") as pspool:

            X = xpool.tile([128, RPC + 2, W + 2, B], f16)
            nc.sync.dma_start(X[0:64], slab_d[0:64])
            nc.scalar.dma_start(X[64:128, 0:RPC], slab_d[64:128, 0:RPC])

            bias_t = bpool.tile([128, LOCS], f32)
            nc.scalar.dma_start(bias_t, bias_d)

            out_rows = {}
            n_groups = NG if ngroups is None else ngroups
            for rep in range(repeat):
              for g in range(n_groups):
                  wp = wpool.tile([128, G * 3, O], f16, tag="wp")
                  ws = spool.tile([64, G * 3, O], f16, tag="ws")
                  ws_eng = nc.scalar if mix == 0 else nc.sync
                  wp_src = wp_d[g].rearrange("p (gk o) -> p gk o", o=O)
                  ws_src = ws_d[g].rearrange("p (gk o) -> p gk o", o=O)
                  if g == n_groups - 1:
                      qg = G * 3 // 4
                      for q in range(4):
                          sl = slice(q * qg, (q + 1) * qg)
                          nc.sync.dma_start(wp[:, sl], wp_src[:, sl])
                          ws_eng.dma_start(ws[:, sl], ws_src[:, sl])
                  elif g == n_groups - 2:
                      hg = G * 3 // 2
                      nc.sync.dma_start(wp[:, 0:hg], wp_src[:, 0:hg])
                      ws_eng.dma_start(ws[:, 0:hg], ws_src[:, 0:hg])
                      nc.sync.dma_start(wp[:, hg:], wp_src[:, hg:])
                      ws_eng.dma_start(ws[:, hg:], ws_src[:, hg:])
                  else:
                      nc.sync.dma_start(wp, wp_src)
                      ws_eng.dma_start(ws, ws_src)

                  for j in range(G):
                      loc = g * G + j
                      hol, wo = divmod(loc, WO)
                      if wo == 0:
                          out_rows[hol] = opool.tile([128, WO, B], f16,
                                                     tag="orow",
                                                     name=f"orow{rep}_{hol}")
                      orow = out_rows[hol]

                      if wo % G == 0:
                          ps = pspool.tile([128, G, B], f32, tag="ps",
                                           name=f"ps{rep}_{loc}")
                      half = ps[:, wo % G, :]
                      for kj in range(3):
                          nc.tensor.matmul(half, wp[:, j * 3 + kj, :],
                                           X[:, hol, wo + kj, :],
                                           start=(kj == 0), stop=False)
                      for kj in range(3):
                          nc.tensor.matmul(half, ws[:, j * 3 + kj, :],
                                           X[0:64, hol + 2, wo + kj, :],
                                           start=False, stop=(kj == 2))
                      if wo % G == G - 1:
                          nc.vector.tensor_tensor(
                              orow[:, wo - (G - 1):wo + 1, :], ps,
                              bias_t[:, loc - (G - 1):loc + 1, None]
                              .to_broadcast((128, G, B)),
                              mybir.AluOpType.add)
                      if hol == RPC - 1 and wo % 16 == 15:
                          nc.sync.dma_start(out_d[hol, :, wo - 15:wo + 1, :],
                                            orow[:, wo - 15:wo + 1, :])
                      elif wo == WO - 1:
                          nc.sync.dma_start(out_d[hol], orow)
    nc.finalize()
    return nc


def _prep_inputs(input, weight, bias):
    inp = np.ascontiguousarray(input, dtype=np.float32)
    wgt = np.ascontiguousarray(weight, dtype=np.float32)
    bis = np.ascontiguousarray(bias, dtype=np.float32)

    in2 = np.ascontiguousarray(inp.transpose(2, 3, 1, 0))        # [h,w,c,b]
    # [ho,wo,kj,(ki01,c)=128,o] and [ho,wo,kj,c,o]
    wp_full = wgt[:, :, :, :, 0:2, :].transpose(0, 1, 5, 4, 3, 2) \
        .reshape(HO, WO, 3, 128, O)
    ws_full = wgt[:, :, :, :, 2, :].transpose(0, 1, 4, 3, 2)

    in_maps = []
    for core in range(NCORES):
        h0 = core * RPC
        # exact SBUF image: [partition, h', w'(padded), b]
        img = np.zeros((128, RPC + 2, W + 2, B), np.float16)
        # lower 64 partitions (c): rows h' = 0..5 <- global rows h0-1..h0+4
        for hp in range(RPC + 2):
            h = h0 - 1 + hp
            if 0 <= h < H:
                img[0:64, hp, 1:W + 1, :] = in2[h].transpose(1, 0, 2)
        # upper 64 partitions: h-shifted copy, h' = 0..3 <- rows h0..h0+3
        for hp in range(RPC):
            img[64:128, hp, 1:W + 1, :] = in2[h0 + hp].transpose(1, 0, 2)
        slab = img
        # [l=(g,j), kj, p, o] -> [g, p, (j, kj, o)] partition-major flat
        wpc = wp_full[h0:h0 + RPC].reshape(NG, G, 3, 128, O)
        wsc = ws_full[h0:h0 + RPC].reshape(NG, G, 3, 64, O)
        in_maps.append({
            "slab": slab,
            "wp": np.ascontiguousarray(wpc.transpose(0, 3, 1, 2, 4))
                .reshape(NG, 128, G * 3 * O).astype(np.float16),
            "ws": np.ascontiguousarray(wsc.transpose(0, 3, 1, 2, 4))
                .reshape(NG, 64, G * 3 * O).astype(np.float16),
            "bias": np.ascontiguousarray(
                bis.reshape(O, HO, WO)[:, h0:h0 + RPC, :].reshape(O, LOCS)),
        })
    return in_maps


_RUN_KW = {}  # test.py can inject trace=True etc.
_LAST_RESULT = [None]
_NC_CACHE = [None]


def kernel(input, weight, bias):
    from concourse.bass_utils import run_bass_kernel_spmd

    in_maps = _prep_inputs(input, weight, bias)
    if _NC_CACHE[0] is None:
        _NC_CACHE[0] = _build_bass()
    nc = _NC_CACHE[0]
    res = run_bass_kernel_spmd(nc, in_maps, core_ids=list(range(NCORES)),
                               **_RUN_KW)
    _LAST_RESULT[0] = res
    arr = np.stack([r["out"] for r in res.results])   # [core,hol,o,wo,b]
    out = arr.astype(np.float32).transpose(4, 2, 0, 1, 3).reshape(B, O, HO, WO)
    return np.ascontiguousarray(out)


# revision 4
# speedup vs baseline: 1.0429x; 1.0429x over previous
"""Local2d (unshared-weight conv) Bass kernel for 8 trn2 NeuronCores.

Problem: input (64,64,32,32), weight (32,32,128,64,3,3), bias (128,32,32)
-> out (64,128,32,32).  K=3, stride 1, pad 1.

Sharding: spatial over h_out — core i handles output rows 4i..4i+3 and
reads the disjoint weight slice for those rows, plus a 6-row input halo
slab.

The kernel is DMA-bound (all traffic serializes through the shared DMA
engines at ~360B/ns), so everything on the wire is fp16: weights, input
slab and the output (PSUM accumulation stays fp32; the host casts the
output back to fp32).  Tolerance is 2e-2; fp16 matmul error is ~5e-4.

DMA-byte trims on top of the fp16 port:
  - matmuls that would multiply an all-zero pad column (wo==0/kj==0 and
    wo==31/kj==2) are skipped entirely, and their weight blocks are not
    shipped (saves 2/96 of weight traffic + the pad columns of the slab);
  - the h-shifted upper-half copy of the input slab is produced on-chip
    by a DVE copy instead of a second HBM load;
  - the weight stream is cut into groups with a tapered tail (8,...,8,
    4,2,1,1 locations) so only a tiny compute chain trails the last
    weight byte.

Per output location (ho,wo) the contraction is over (c,ki,kj) = 576,
packed as up to 6 PE matmuls accumulating in PSUM:
  - "paired" matmuls, K=128: partitions 0-63 = channels at ki=0,
    partitions 64-127 = channels at ki=1 (the upper 64 partitions hold a
    copy of the slab shifted by one input row, so a single access-pattern
    offset addresses both ki rows);
  - "single" matmuls, K=64: channels at ki=2.
Stationary operand = per-location weights [K,128(o)], moving = input
columns [K,64(b)].  Host pre-transposes the weights so the contraction
dim lands on partitions with fully contiguous DMA.
"""

import numpy as np

B, C, O, KK, H, W = 64, 64, 128, 3, 32, 32
HO = WO = 32
NCORES = 8
RPC = HO // NCORES          # output rows per core
LOCS = RPC * WO             # locations per core

# location groups: one weight DMA (wp+ws) per group, tapered at the end
_GROUP_SIZES = [8] * 15 + [4, 2, 1, 1]


def _skip(wo, kj):
    return (wo == 0 and kj == 0) or (wo == WO - 1 and kj == 2)


def _group_plan():
    """[(loc_start, nlocs, [(loc, kj), ...]), ...] with block order shared
    by the host weight packer and the kernel builder."""
    plan, s = [], 0
    for n in _GROUP_SIZES:
        blks = []
        for loc in range(s, s + n):
            wo = loc % WO
            for kj in range(3):
                if not _skip(wo, kj):
                    blks.append((loc, kj))
        plan.append((s, n, blks))
        s += n
    assert s == LOCS
    return plan


_PLAN = _group_plan()
NBLK = sum(len(b) for _, _, b in _PLAN)

# bias-add / PSUM granularity: segment end -> (width, psum tag)
_SEG_END = {7: 8, 15: 8, 23: 8, 27: 4, 31: 4}   # row 3 uses 27/31 splits
# output DMA chunks for the last row: wo -> chunk width
_OUT_CHUNK = {15: 16, 23: 8, 27: 4, 31: 4}


def _build_bass(mode="full", ngroups=None, mix=0, repeat=1):
    from concourse import bacc
    import concourse.mybir as mybir
    from concourse.tile import TileContext

    f16 = mybir.dt.float16
    f32 = mybir.dt.float32
    nc = bacc.Bacc("TRN2", target_bir_lowering=False, debug=False,
                   num_devices=NCORES)

    # input slab without pad columns: [64, 6, 32, 64] (partition-major)
    slab_d = nc.dram_tensor("slab", (64, RPC + 2, W, B), f16,
                            kind="ExternalInput").ap()
    # weights pre-arranged on host as one partition-major flat stream;
    # per-group slices are fully contiguous per partition.
    wp_d = nc.dram_tensor("wp", (128, NBLK * O), f16,
                          kind="ExternalInput").ap()
    ws_d = nc.dram_tensor("ws", (64, NBLK * O), f16,
                          kind="ExternalInput").ap()
    bias_d = nc.dram_tensor("bias", (O, LOCS), f32,
                            kind="ExternalInput").ap()
    out_d = nc.dram_tensor("out", (RPC, O, WO, B), f16,
                           kind="ExternalOutput").ap()

    with TileContext(nc) as tc:
        with tc.tile_pool(name="xslab", bufs=1) as xpool, \
             tc.tile_pool(name="wpool", bufs=4) as wpool, \
             tc.tile_pool(name="spool", bufs=4) as spool, \
             tc.tile_pool(name="bpool", bufs=1) as bpool, \
             tc.tile_pool(name="opool", bufs=2) as opool, \
             tc.tile_pool(name="psum", bufs=4, space="PSUM") as pspool:

            X = xpool.tile([128, RPC + 2, W + 2, B], f16)
            nc.sync.dma_start(X[0:64, :, 1:W + 1, :], slab_d)
            # h-shifted upper-half copy, on-chip (pad cols never read)
            nc.vector.tensor_copy(X[64:128, 0:RPC, 1:W + 1, :],
                                  X[0:64, 1:RPC + 1, 1:W + 1, :])

            bias_t = bpool.tile([128, LOCS], f32)
            nc.scalar.dma_start(bias_t, bias_d)

            out_rows = {}
            groups = _PLAN if ngroups is None else _PLAN[:ngroups]
            for rep in range(repeat):
              off = 0
              for gi, (s, n, blks) in enumerate(groups):
                  nb = len(blks)
                  bidx = {lk: i for i, lk in enumerate(blks)}
                  wp = wpool.tile([128, nb, O], f16, tag="wp",
                                  name=f"wp{rep}_{gi}")
                  ws = spool.tile([64, nb, O], f16, tag="ws",
                                  name=f"ws{rep}_{gi}")
                  wp_src = wp_d[:, off * O:(off + nb) * O] \
                      .rearrange("p (n o) -> p n o", o=O)
                  ws_src = ws_d[:, off * O:(off + nb) * O] \
                      .rearrange("p (n o) -> p n o", o=O)
                  nc.sync.dma_start(wp, wp_src)
                  (nc.scalar if mix == 0 else nc.sync).dma_start(ws, ws_src)
                  off += nb

                  for loc in range(s, s + n):
                      hol, wo = divmod(loc, WO)
                      if wo == 0:
                          out_rows[hol] = opool.tile([128, WO, B], f16,
                                                     tag="orow",
                                                     name=f"orow{rep}_{hol}")
                      orow = out_rows[hol]

                      last_row = hol == RPC - 1
                      seg_w = 4 if (last_row and wo >= 24) else 8
                      if (wo % 8 == 0 and not (last_row and wo >= 24)) or \
                              (last_row and wo in (24, 28)):
                          ps = pspool.tile([128, seg_w, B], f32,
                                           tag=f"ps{seg_w}",
                                           name=f"ps{rep}_{loc}")
                          seg0 = wo
                      kjs = [kj for kj in range(3) if not _skip(wo, kj)]
                      half = ps[:, wo - seg0, :]
                      for i, kj in enumerate(kjs):
                          nc.tensor.matmul(half, wp[:, bidx[(loc, kj)], :],
                                           X[:, hol, wo + kj, :],
                                           start=(i == 0), stop=False)
                      for i, kj in enumerate(kjs):
                          nc.tensor.matmul(half, ws[:, bidx[(loc, kj)], :],
                                           X[0:64, hol + 2, wo + kj, :],
                                           start=False,
                                           stop=(i == len(kjs) - 1))
                      seg_end = (wo - seg0 == seg_w - 1)
                      if seg_end:
                          nc.vector.tensor_tensor(
                              orow[:, seg0:wo + 1, :], ps,
                              bias_t[:, loc - (seg_w - 1):loc + 1, None]
                              .to_broadcast((128, seg_w, B)),
                              mybir.AluOpType.add)
                      if last_row:
                          cw = _OUT_CHUNK.get(wo)
                          if cw is not None:
                              nc.sync.dma_start(
                                  out_d[hol, :, wo - cw + 1:wo + 1, :],
                                  orow[:, wo - cw + 1:wo + 1, :])
                      elif wo == WO - 1:
                          nc.sync.dma_start(out_d[hol], orow)
    nc.finalize()
    return nc


def _prep_inputs(input, weight, bias):
    inp = np.ascontiguousarray(input, dtype=np.float32)
    wgt = np.ascontiguousarray(weight, dtype=np.float32)
    bis = np.ascontiguousarray(bias, dtype=np.float32)

    in2 = np.ascontiguousarray(inp.transpose(2, 3, 1, 0))        # [h,w,c,b]
    # [ho,wo,kj,(ki01,c)=128,o] and [ho,wo,kj,c,o]
    wp_full = wgt[:, :, :, :, 0:2, :].transpose(0, 1, 5, 4, 3, 2) \
        .reshape(HO, WO, 3, 128, O).astype(np.float16)
    ws_full = wgt[:, :, :, :, 2, :].transpose(0, 1, 4, 3, 2) \
        .astype(np.float16)

    in_maps = []
    for core in range(NCORES):
        h0 = core * RPC
        # slab image without pad columns: [c=64, h', w, b]
        img = np.zeros((64, RPC + 2, W, B), np.float16)
        for hp in range(RPC + 2):
            h = h0 - 1 + hp
            if 0 <= h < H:
                img[:, hp] = in2[h].transpose(1, 0, 2)
        wp_blocks = []
        ws_blocks = []
        for s, n, blks in _PLAN:
            for loc, kj in blks:
                hol, wo = divmod(loc, WO)
                wp_blocks.append(wp_full[h0 + hol, wo, kj])   # [128, O]
                ws_blocks.append(ws_full[h0 + hol, wo, kj])   # [64, O]
        wp_c = np.stack(wp_blocks)          # [NBLK, 128, O]
        ws_c = np.stack(ws_blocks)          # [NBLK, 64, O]
        in_maps.append({
            "slab": img,
            "wp": np.ascontiguousarray(wp_c.transpose(1, 0, 2))
                .reshape(128, NBLK * O),
            "ws": np.ascontiguousarray(ws_c.transpose(1, 0, 2))
                .reshape(64, NBLK * O),
            "bias": np.ascontiguousarray(
                bis.reshape(O, HO, WO)[:, h0:h0 + RPC, :].reshape(O, LOCS)),
        })
    return in_maps


_RUN_KW = {}  # test.py can inject trace=True etc.
_LAST_RESULT = [None]
_NC_CACHE = [None]


def kernel(input, weight, bias):
    from concourse.bass_utils import run_bass_kernel_spmd

    in_maps = _prep_inputs(input, weight, bias)
    if _NC_CACHE[0] is None:
        _NC_CACHE[0] = _build_bass()
    nc = _NC_CACHE[0]
    res = run_bass_kernel_spmd(nc, in_maps, core_ids=list(range(NCORES)),
                               **_RUN_KW)
    _LAST_RESULT[0] = res
    arr = np.stack([r["out"] for r in res.results])   # [core,hol,o,wo,b]
    out = arr.astype(np.float32).transpose(4, 2, 0, 1, 3).reshape(B, O, HO, WO)
    return np.ascontiguousarray(out)


# revision 7
# speedup vs baseline: 1.0965x; 1.0513x over previous
"""Local2d (unshared-weight conv) Bass kernel for 8 trn2 NeuronCores.

Problem: input (64,64,32,32), weight (32,32,128,64,3,3), bias (128,32,32)
-> out (64,128,32,32).  K=3, stride 1, pad 1.

Sharding: spatial over h_out — core i handles output rows 4i..4i+3 and
reads the disjoint weight slice for those rows, plus a 6-row input halo
slab.

The kernel is DMA-bound (all traffic serializes through the shared DMA
engines at ~360B/ns), so everything on the wire is fp16: weights, input
slab and the output (PSUM accumulation stays fp32; the host casts the
output back to fp32).  Tolerance is 2e-2; fp16 matmul error is ~5e-4.

DMA-byte trims on top of the fp16 port:
  - matmuls that would multiply an all-zero pad column (wo==0/kj==0 and
    wo==31/kj==2) are skipped entirely, and their weight blocks are not
    shipped (saves 2/96 of weight traffic + the pad columns of the slab);
  - the h-shifted upper-half copy of the input slab is produced on-chip
    by a DVE copy instead of a second HBM load;
  - the weight stream is cut into groups with a tapered tail (8,...,8,
    4,2,1,1 locations) so only a tiny compute chain trails the last
    weight byte.

Per output location (ho,wo) the contraction is over (c,ki,kj) = 576,
packed as up to 6 PE matmuls accumulating in PSUM:
  - "paired" matmuls, K=128: partitions 0-63 = channels at ki=0,
    partitions 64-127 = channels at ki=1 (the upper 64 partitions hold a
    copy of the slab shifted by one input row, so a single access-pattern
    offset addresses both ki rows);
  - "single" matmuls, K=64: channels at ki=2.
Stationary operand = per-location weights [K,128(o)], moving = input
columns [K,64(b)].  Host pre-transposes the weights so the contraction
dim lands on partitions with fully contiguous DMA.
"""

import numpy as np

B, C, O, KK, H, W = 64, 64, 128, 3, 32, 32
HO = WO = 32
NCORES = 8
RPC = HO // NCORES          # output rows per core
LOCS = RPC * WO             # locations per core

# location groups: one weight DMA (wp+ws) per group, tapered at the end
_GROUP_SIZES = [8] * 15 + [4, 2, 1, 1]


def _skip(wo, kj):
    return (wo == 0 and kj == 0) or (wo == WO - 1 and kj == 2)


def _group_plan():
    """[(loc_start, nlocs, [(loc, kj), ...]), ...] with block order shared
    by the host weight packer and the kernel builder."""
    plan, s = [], 0
    for n in _GROUP_SIZES:
        blks = []
        for loc in range(s, s + n):
            wo = loc % WO
            for kj in range(3):
                if not _skip(wo, kj):
                    blks.append((loc, kj))
        plan.append((s, n, blks))
        s += n
    assert s == LOCS
    return plan


_PLAN = _group_plan()
NBLK = sum(len(b) for _, _, b in _PLAN)

# bias-add / PSUM granularity: segment end -> (width, psum tag)
_SEG_END = {7: 8, 15: 8, 23: 8, 27: 4, 31: 4}   # row 3 uses 27/31 splits
# output DMA chunks for the last row: wo -> chunk width
_OUT_CHUNK = {15: 16, 23: 8, 27: 4, 31: 4}


def _build_bass(mode="full", ngroups=None, mix=0, repeat=1):
    from concourse import bacc
    import concourse.mybir as mybir
    from concourse.tile import TileContext

    f16 = mybir.dt.float16
    f32 = mybir.dt.float32
    nc = bacc.Bacc("TRN2", target_bir_lowering=False, debug=False,
                   num_devices=NCORES)

    # input slab without pad columns: [64, 6, 32, 64] (partition-major)
    slab_d = nc.dram_tensor("slab", (64, RPC + 2, W, B), f16,
                            kind="ExternalInput").ap()
    # weights pre-arranged on host as one partition-major flat stream;
    # per-group slices are fully contiguous per partition.
    wp_d = nc.dram_tensor("wp", (128, NBLK * O), f16,
                          kind="ExternalInput").ap()
    ws_d = nc.dram_tensor("ws", (64, NBLK * O), f16,
                          kind="ExternalInput").ap()
    bias_d = nc.dram_tensor("bias", (O, LOCS), f32,
                            kind="ExternalInput").ap()
    out_d = nc.dram_tensor("out", (RPC, O, WO, B), f16,
                           kind="ExternalOutput").ap()

    with TileContext(nc) as tc:
        with tc.tile_pool(name="xslab", bufs=1) as xpool, \
             tc.tile_pool(name="wpool", bufs=8) as wpool, \
             tc.tile_pool(name="spool", bufs=8) as spool, \
             tc.tile_pool(name="bpool", bufs=1) as bpool, \
             tc.tile_pool(name="opool", bufs=2) as opool, \
             tc.tile_pool(name="psum", bufs=4, space="PSUM") as pspool:

            X = xpool.tile([128, RPC + 2, W + 2, B], f16)
            nc.sync.dma_start(X[0:64, :, 1:W + 1, :], slab_d)
            # h-shifted upper-half copy, on-chip (pad cols never read)
            nc.vector.tensor_copy(X[64:128, 0:RPC, 1:W + 1, :],
                                  X[0:64, 1:RPC + 1, 1:W + 1, :])

            bias_t = bpool.tile([128, LOCS], f32)
            nc.scalar.dma_start(bias_t, bias_d)

            out_rows = {}
            groups = _PLAN if ngroups is None else _PLAN[:ngroups]
            for rep in range(repeat):
              off = 0
              for gi, (s, n, blks) in enumerate(groups):
                  nb = len(blks)
                  bidx = {lk: i for i, lk in enumerate(blks)}
                  wp = wpool.tile([128, nb, O], f16, tag="wp",
                                  name=f"wp{rep}_{gi}")
                  ws = spool.tile([64, nb, O], f16, tag="ws",
                                  name=f"ws{rep}_{gi}")
                  wp_src = wp_d[:, off * O:(off + nb) * O] \
                      .rearrange("p (n o) -> p n o", o=O)
                  ws_src = ws_d[:, off * O:(off + nb) * O] \
                      .rearrange("p (n o) -> p n o", o=O)
                  nc.sync.dma_start(wp, wp_src)
                  (nc.scalar if mix == 0 else nc.sync).dma_start(ws, ws_src)
                  off += nb

                  for loc in range(s, s + n):
                      hol, wo = divmod(loc, WO)
                      if wo == 0:
                          out_rows[hol] = opool.tile([128, WO, B], f16,
                                                     tag="orow",
                                                     name=f"orow{rep}_{hol}")
                      orow = out_rows[hol]

                      last_row = hol == RPC - 1
                      seg_w = 4 if (last_row and wo >= 24) else 8
                      if (wo % 8 == 0 and not (last_row and wo >= 24)) or \
                              (last_row and wo in (24, 28)):
                          ps = pspool.tile([128, seg_w, B], f32,
                                           tag=f"ps{seg_w}",
                                           name=f"ps{rep}_{loc}")
                          seg0 = wo
                      kjs = [kj for kj in range(3) if not _skip(wo, kj)]
                      half = ps[:, wo - seg0, :]
                      for i, kj in enumerate(kjs):
                          nc.tensor.matmul(half, wp[:, bidx[(loc, kj)], :],
                                           X[:, hol, wo + kj, :],
                                           start=(i == 0), stop=False)
                      for i, kj in enumerate(kjs):
                          nc.tensor.matmul(half, ws[:, bidx[(loc, kj)], :],
                                           X[0:64, hol + 2, wo + kj, :],
                                           start=False,
                                           stop=(i == len(kjs) - 1))
                      seg_end = (wo - seg0 == seg_w - 1)
                      if seg_end:
                          nc.vector.tensor_tensor(
                              orow[:, seg0:wo + 1, :], ps,
                              bias_t[:, loc - (seg_w - 1):loc + 1, None]
                              .to_broadcast((128, seg_w, B)),
                              mybir.AluOpType.add)
                      # output DMAs ride the idle gpsimd (SWDGE) queue so
                      # their compute-dependent waits never block the weight
                      # streams on sync/scalar; the final chunk goes on sync,
                      # which is idle by then and has the shortest issue path.
                      if last_row:
                          cw = _OUT_CHUNK.get(wo)
                          if cw is not None:
                              eng = nc.sync if wo == WO - 1 else nc.gpsimd
                              eng.dma_start(
                                  out_d[hol, :, wo - cw + 1:wo + 1, :],
                                  orow[:, wo - cw + 1:wo + 1, :])
                      elif wo == WO - 1:
                          nc.gpsimd.dma_start(out_d[hol], orow)
    nc.finalize()
    return nc


def _prep_inputs(input, weight, bias):
    inp = np.ascontiguousarray(input, dtype=np.float32)
    wgt = np.ascontiguousarray(weight, dtype=np.float32)
    bis = np.ascontiguousarray(bias, dtype=np.float32)

    in2 = np.ascontiguousarray(inp.transpose(2, 3, 1, 0))        # [h,w,c,b]
    # [ho,wo,kj,(ki01,c)=128,o] and [ho,wo,kj,c,o]
    wp_full = wgt[:, :, :, :, 0:2, :].transpose(0, 1, 5, 4, 3, 2) \
        .reshape(HO, WO, 3, 128, O).astype(np.float16)
    ws_full = wgt[:, :, :, :, 2, :].transpose(0, 1, 4, 3, 2) \
        .astype(np.float16)

    in_maps = []
    for core in range(NCORES):
        h0 = core * RPC
        # slab image without pad columns: [c=64, h', w, b]
        img = np.zeros((64, RPC + 2, W, B), np.float16)
        for hp in range(RPC + 2):
            h = h0 - 1 + hp
            if 0 <= h < H:
                img[:, hp] = in2[h].transpose(1, 0, 2)
        wp_blocks = []
        ws_blocks = []
        for s, n, blks in _PLAN:
            for loc, kj in blks:
                hol, wo = divmod(loc, WO)
                wp_blocks.append(wp_full[h0 + hol, wo, kj])   # [128, O]
                ws_blocks.append(ws_full[h0 + hol, wo, kj])   # [64, O]
        wp_c = np.stack(wp_blocks)          # [NBLK, 128, O]
        ws_c = np.stack(ws_blocks)          # [NBLK, 64, O]
        in_maps.append({
            "slab": img,
            "wp": np.ascontiguousarray(wp_c.transpose(1, 0, 2))
                .reshape(128, NBLK * O),
            "ws": np.ascontiguousarray(ws_c.transpose(1, 0, 2))
                .reshape(64, NBLK * O),
            "bias": np.ascontiguousarray(
                bis.reshape(O, HO, WO)[:, h0:h0 + RPC, :].reshape(O, LOCS)),
        })
    return in_maps


_RUN_KW = {}  # test.py can inject trace=True etc.
_LAST_RESULT = [None]
_NC_CACHE = [None]


def kernel(input, weight, bias):
    from concourse.bass_utils import run_bass_kernel_spmd

    in_maps = _prep_inputs(input, weight, bias)
    if _NC_CACHE[0] is None:
        _NC_CACHE[0] = _build_bass()
    nc = _NC_CACHE[0]
    res = run_bass_kernel_spmd(nc, in_maps, core_ids=list(range(NCORES)),
                               **_RUN_KW)
    _LAST_RESULT[0] = res
    arr = np.stack([r["out"] for r in res.results])   # [core,hol,o,wo,b]
    out = arr.astype(np.float32).transpose(4, 2, 0, 1, 3).reshape(B, O, HO, WO)
    return np.ascontiguousarray(out)


# revision 11
# speedup vs baseline: 1.1003x; 1.0035x over previous
"""Local2d (unshared-weight conv) Bass kernel for 8 trn2 NeuronCores.

Problem: input (64,64,32,32), weight (32,32,128,64,3,3), bias (128,32,32)
-> out (64,128,32,32).  K=3, stride 1, pad 1.

Sharding: spatial over h_out — core i handles output rows 4i..4i+3 and
reads the disjoint weight slice for those rows, plus a 6-row input halo
slab.

The kernel is DMA-bound (all traffic serializes through the shared DMA
engines at ~360B/ns), so everything on the wire is fp16: weights, input
slab and the output (PSUM accumulation stays fp32; the host casts the
output back to fp32).  Tolerance is 2e-2; fp16 matmul error is ~5e-4.

DMA-byte trims on top of the fp16 port:
  - matmuls that would multiply an all-zero pad column (wo==0/kj==0 and
    wo==31/kj==2) are skipped entirely, and their weight blocks are not
    shipped (saves 2/96 of weight traffic + the pad columns of the slab);
  - the h-shifted upper-half copy of the input slab is produced on-chip
    by a DVE copy instead of a second HBM load;
  - the weight stream is cut into groups with a tapered tail (8,...,8,
    4,2,1,1 locations) so only a tiny compute chain trails the last
    weight byte.

Per output location (ho,wo) the contraction is over (c,ki,kj) = 576,
packed as up to 6 PE matmuls accumulating in PSUM:
  - "paired" matmuls, K=128: partitions 0-63 = channels at ki=0,
    partitions 64-127 = channels at ki=1 (the upper 64 partitions hold a
    copy of the slab shifted by one input row, so a single access-pattern
    offset addresses both ki rows);
  - "single" matmuls, K=64: channels at ki=2.
Stationary operand = per-location weights [K,128(o)], moving = input
columns [K,64(b)].  Host pre-transposes the weights so the contraction
dim lands on partitions with fully contiguous DMA.
"""

import numpy as np

B, C, O, KK, H, W = 64, 64, 128, 3, 32, 32
HO = WO = 32
NCORES = 8
RPC = HO // NCORES          # output rows per core
LOCS = RPC * WO             # locations per core

# location groups: one weight DMA (wp+ws) per group, tapered at the end
_GROUP_SIZES = [8] * 15 + [4, 2, 1, 1]


def _skip(wo, kj):
    return (wo == 0 and kj == 0) or (wo == WO - 1 and kj == 2)


def _group_plan():
    """[(loc_start, nlocs, [(loc, kj), ...]), ...] with block order shared
    by the host weight packer and the kernel builder."""
    plan, s = [], 0
    for n in _GROUP_SIZES:
        blks = []
        for loc in range(s, s + n):
            wo = loc % WO
            for kj in range(3):
                if not _skip(wo, kj):
                    blks.append((loc, kj))
        plan.append((s, n, blks))
        s += n
    assert s == LOCS
    return plan


_PLAN = _group_plan()
NBLK = sum(len(b) for _, _, b in _PLAN)

# last row: PSUM/bias-add segment starts -> width (rows 0-2 use 8 wide)
_SEG_START = {0: 8, 8: 8, 16: 8, 24: 4, 28: 2, 30: 2}
# output DMA chunks for the last row: wo -> chunk width
_OUT_CHUNK = {15: 16, 23: 8, 27: 4, 29: 2, 31: 2}


def _build_bass(mode="full", ngroups=None, mix=0, repeat=1):
    from concourse import bacc
    import concourse.mybir as mybir
    from concourse.tile import TileContext

    f16 = mybir.dt.float16
    f32 = mybir.dt.float32
    nc = bacc.Bacc("TRN2", target_bir_lowering=False, debug=False,
                   num_devices=NCORES)

    # input slab without pad columns: [64, 6, 32, 64] (partition-major)
    slab_d = nc.dram_tensor("slab", (64, RPC + 2, W, B), f16,
                            kind="ExternalInput").ap()
    # weights pre-arranged on host as one partition-major flat stream;
    # per-group slices are fully contiguous per partition.
    wp_d = nc.dram_tensor("wp", (128, NBLK * O), f16,
                          kind="ExternalInput").ap()
    ws_d = nc.dram_tensor("ws", (64, NBLK * O), f16,
                          kind="ExternalInput").ap()
    bias_d = nc.dram_tensor("bias", (O, LOCS), f32,
                            kind="ExternalInput").ap()
    out_d = nc.dram_tensor("out", (RPC, O, WO, B), f16,
                           kind="ExternalOutput").ap()

    with TileContext(nc) as tc:
        with tc.tile_pool(name="xslab", bufs=1) as xpool, \
             tc.tile_pool(name="wpool", bufs=8) as wpool, \
             tc.tile_pool(name="spool", bufs=8) as spool, \
             tc.tile_pool(name="bpool", bufs=1) as bpool, \
             tc.tile_pool(name="opool", bufs=2) as opool, \
             tc.tile_pool(name="psum", bufs=4, space="PSUM") as pspool:

            X = xpool.tile([128, RPC + 2, W + 2, B], f16)
            nc.sync.dma_start(X[0:64, :, 1:W + 1, :], slab_d)
            # h-shifted upper-half copy, on-chip (pad cols never read)
            nc.vector.tensor_copy(X[64:128, 0:RPC, 1:W + 1, :],
                                  X[0:64, 1:RPC + 1, 1:W + 1, :])

            bias_t = bpool.tile([128, LOCS], f32)
            nc.scalar.dma_start(bias_t, bias_d)

            out_rows = {}
            groups = _PLAN if ngroups is None else _PLAN[:ngroups]
            for rep in range(repeat):
              off = 0
              for gi, (s, n, blks) in enumerate(groups):
                  nb = len(blks)
                  bidx = {lk: i for i, lk in enumerate(blks)}
                  wp = wpool.tile([128, nb, O], f16, tag="wp",
                                  name=f"wp{rep}_{gi}")
                  ws = spool.tile([64, nb, O], f16, tag="ws",
                                  name=f"ws{rep}_{gi}")
                  wp_src = wp_d[:, off * O:(off + nb) * O] \
                      .rearrange("p (n o) -> p n o", o=O)
                  ws_src = ws_d[:, off * O:(off + nb) * O] \
                      .rearrange("p (n o) -> p n o", o=O)
                  nc.sync.dma_start(wp, wp_src)
                  (nc.scalar if mix == 0 else nc.sync).dma_start(ws, ws_src)
                  off += nb

                  for loc in range(s, s + n):
                      hol, wo = divmod(loc, WO)
                      if wo == 0:
                          out_rows[hol] = opool.tile([128, WO, B], f16,
                                                     tag="orow",
                                                     name=f"orow{rep}_{hol}")
                      orow = out_rows[hol]

                      last_row = hol == RPC - 1
                      if last_row:
                          if wo in _SEG_START:
                              seg_w = _SEG_START[wo]
                              seg0 = wo
                      elif wo % 8 == 0:
                          seg_w, seg0 = 8, wo
                      if wo == seg0:
                          ps = pspool.tile([128, seg_w, B], f32,
                                           tag=f"ps{seg_w}", bufs=4 if seg_w == 8 else 2,
                                           name=f"ps{rep}_{loc}")
                      kjs = [kj for kj in range(3) if not _skip(wo, kj)]
                      half = ps[:, wo - seg0, :]
                      for i, kj in enumerate(kjs):
                          nc.tensor.matmul(half, wp[:, bidx[(loc, kj)], :],
                                           X[:, hol, wo + kj, :],
                                           start=(i == 0), stop=False)
                      for i, kj in enumerate(kjs):
                          nc.tensor.matmul(half, ws[:, bidx[(loc, kj)], :],
                                           X[0:64, hol + 2, wo + kj, :],
                                           start=False,
                                           stop=(i == len(kjs) - 1))
                      seg_end = (wo - seg0 == seg_w - 1)
                      if seg_end:
                          nc.vector.tensor_tensor(
                              orow[:, seg0:wo + 1, :], ps,
                              bias_t[:, loc - (seg_w - 1):loc + 1, None]
                              .to_broadcast((128, seg_w, B)),
                              mybir.AluOpType.add)
                      # output DMAs ride the idle gpsimd (SWDGE) queue so
                      # their compute-dependent waits never block the weight
                      # streams on sync/scalar; the final chunks spread over
                      # scalar/sync (both idle by then) so the Pool desc-gen
                      # FIFO doesn't serialize the tail.
                      if last_row:
                          cw = _OUT_CHUNK.get(wo)
                          if cw is not None:
                              eng = {27: nc.scalar, 29: nc.scalar,
                                     31: nc.sync}.get(wo, nc.gpsimd)
                              eng.dma_start(
                                  out_d[hol, :, wo - cw + 1:wo + 1, :],
                                  orow[:, wo - cw + 1:wo + 1, :])
                      elif wo == WO - 1:
                          nc.gpsimd.dma_start(out_d[hol], orow)
    nc.finalize()
    return nc


def _prep_inputs(input, weight, bias):
    inp = np.ascontiguousarray(input, dtype=np.float32)
    wgt = np.ascontiguousarray(weight, dtype=np.float32)
    bis = np.ascontiguousarray(bias, dtype=np.float32)

    in2 = np.ascontiguousarray(inp.transpose(2, 3, 1, 0))        # [h,w,c,b]
    # [ho,wo,kj,(ki01,c)=128,o] and [ho,wo,kj,c,o]
    wp_full = wgt[:, :, :, :, 0:2, :].transpose(0, 1, 5, 4, 3, 2) \
        .reshape(HO, WO, 3, 128, O).astype(np.float16)
    ws_full = wgt[:, :, :, :, 2, :].transpose(0, 1, 4, 3, 2) \
        .astype(np.float16)

    in_maps = []
    for core in range(NCORES):
        h0 = core * RPC
        # slab image without pad columns: [c=64, h', w, b]
        img = np.zeros((64, RPC + 2, W, B), np.float16)
        for hp in range(RPC + 2):
            h = h0 - 1 + hp
            if 0 <= h < H:
                img[:, hp] = in2[h].transpose(1, 0, 2)
        wp_blocks = []
        ws_blocks = []
        for s, n, blks in _PLAN:
            for loc, kj in blks:
                hol, wo = divmod(loc, WO)
                wp_blocks.append(wp_full[h0 + hol, wo, kj])   # [128, O]
                ws_blocks.append(ws_full[h0 + hol, wo, kj])   # [64, O]
        wp_c = np.stack(wp_blocks)          # [NBLK, 128, O]
        ws_c = np.stack(ws_blocks)          # [NBLK, 64, O]
        in_maps.append({
            "slab": img,
            "wp": np.ascontiguousarray(wp_c.transpose(1, 0, 2))
                .reshape(128, NBLK * O),
            "ws": np.ascontiguousarray(ws_c.transpose(1, 0, 2))
                .reshape(64, NBLK * O),
            "bias": np.ascontiguousarray(
                bis.reshape(O, HO, WO)[:, h0:h0 + RPC, :].reshape(O, LOCS)),
        })
    return in_maps


_RUN_KW = {}  # test.py can inject trace=True etc.
_LAST_RESULT = [None]
_NC_CACHE = [None]


def kernel(input, weight, bias):
    from concourse.bass_utils import run_bass_kernel_spmd

    in_maps = _prep_inputs(input, weight, bias)
    if _NC_CACHE[0] is None:
        _NC_CACHE[0] = _build_bass()
    nc = _NC_CACHE[0]
    res = run_bass_kernel_spmd(nc, in_maps, core_ids=list(range(NCORES)),
                               **_RUN_KW)
    _LAST_RESULT[0] = res
    arr = np.stack([r["out"] for r in res.results])   # [core,hol,o,wo,b]
    out = arr.astype(np.float32).transpose(4, 2, 0, 1, 3).reshape(B, O, HO, WO)
    return np.ascontiguousarray(out)


# revision 19
# speedup vs baseline: 1.2513x; 1.1372x over previous
"""Local2d (unshared-weight conv) Bass kernel for 8 trn2 NeuronCores.

Problem: input (64,64,32,32), weight (32,32,128,64,3,3), bias (128,32,32)
-> out (64,128,32,32).  K=3, stride 1, pad 1.

Sharding: spatial over h_out — core i handles output rows 4i..4i+3 and
reads the disjoint weight slice for those rows, plus a 6-row input halo
slab.

The kernel is DMA-bound (all traffic serializes through the shared DMA
engines at ~360B/ns), so everything on the wire is fp16: weights, input
slab and the output (PSUM accumulation stays fp32; the host casts the
output back to fp32).  Tolerance is 2e-2; fp16 matmul error is ~5e-4.

DMA-byte trims on top of the fp16 port:
  - matmuls that would multiply an all-zero pad column (wo==0/kj==0 and
    wo==31/kj==2) are skipped entirely, and their weight blocks are not
    shipped (saves 2/96 of weight traffic + the pad columns of the slab);
  - the h-shifted upper-half copy of the input slab is produced on-chip
    by a DVE copy instead of a second HBM load;
  - the weight stream is cut into groups with a tapered tail (8,...,8,
    4,2,1,1 locations) so only a tiny compute chain trails the last
    weight byte.

Per output location (ho,wo) the contraction is over (c,ki,kj) = 576,
packed as up to 6 PE matmuls accumulating in PSUM:
  - "paired" matmuls, K=128: partitions 0-63 = channels at ki=0,
    partitions 64-127 = channels at ki=1 (the upper 64 partitions hold a
    copy of the slab shifted by one input row, so a single access-pattern
    offset addresses both ki rows);
  - "single" matmuls, K=64: channels at ki=2.
Stationary operand = per-location weights [K,128(o)], moving = input
columns [K,64(b)].  Host pre-transposes the weights so the contraction
dim lands on partitions with fully contiguous DMA.
"""

import numpy as np

B, C, O, KK, H, W = 64, 64, 128, 3, 32, 32
HO = WO = 32
NCORES = 8
RPC = HO // NCORES          # output rows per core
LOCS = RPC * WO             # locations per core

# location groups: one weight DMA (wp+ws) per group, tapered at the end
_GROUP_SIZES = [8] * 15 + [4, 2, 1, 1]


def _skip(wo, kj):
    return (wo == 0 and kj == 0) or (wo == WO - 1 and kj == 2)


def _group_plan():
    """[(loc_start, nlocs, [(loc, kj), ...]), ...] with block order shared
    by the host weight packer and the kernel builder."""
    plan, s = [], 0
    for n in _GROUP_SIZES:
        blks = []
        for loc in range(s, s + n):
            wo = loc % WO
            for kj in range(3):
                if not _skip(wo, kj):
                    blks.append((loc, kj))
        plan.append((s, n, blks))
        s += n
    assert s == LOCS
    return plan


_PLAN = _group_plan()
NBLK = sum(len(b) for _, _, b in _PLAN)

# last row: PSUM/bias-add segment starts -> width (rows 0-2 use 8 wide)
_SEG_START = {0: 8, 8: 8, 16: 8, 24: 4, 28: 2, 30: 2}
# output DMA chunks for the last row: wo -> chunk width
_OUT_CHUNK = {15: 16, 23: 8, 27: 4, 31: 4}


def _build_bass(mode="full", ngroups=None, mix=0, repeat=1):
    from concourse import bacc
    import concourse.mybir as mybir
    from concourse.tile import TileContext

    f16 = mybir.dt.float16
    f32 = mybir.dt.float32
    f8 = mybir.dt.float8e4
    nc = bacc.Bacc("TRN2", target_bir_lowering=False, debug=False,
                   num_devices=NCORES)

    # input slab without pad columns: [64, 6, 32, 64] (partition-major)
    slab_d = nc.dram_tensor("slab", (64, RPC + 2, W, B), f16,
                            kind="ExternalInput").ap()
    # weights pre-arranged on host as one partition-major flat stream;
    # per-group slices are fully contiguous per partition.  The ki=2
    # "single" weights travel as fp8 e4m3 (measured rel err 1.6e-2 vs the
    # 2e-2 budget), halving that stream.
    wp_d = nc.dram_tensor("wp", (128, NBLK * O), f16,
                          kind="ExternalInput").ap()
    ws_d = nc.dram_tensor("ws", (64, NBLK * O), f8,
                          kind="ExternalInput").ap()
    bias_d = nc.dram_tensor("bias", (O, LOCS), f16,
                            kind="ExternalInput").ap()
    out_d = nc.dram_tensor("out", (RPC, O, WO, B), f16,
                           kind="ExternalOutput").ap()

    with TileContext(nc) as tc:
        with tc.tile_pool(name="xslab", bufs=1) as xpool, \
             tc.tile_pool(name="wpool", bufs=8) as wpool, \
             tc.tile_pool(name="spool", bufs=8) as spool, \
             tc.tile_pool(name="bpool", bufs=1) as bpool, \
             tc.tile_pool(name="opool", bufs=2) as opool, \
             tc.tile_pool(name="psum", bufs=4, space="PSUM") as pspool:

            X = xpool.tile([128, RPC + 2, W + 2, B], f16)
            nc.sync.dma_start(X[0:64, :, 1:W + 1, :], slab_d)
            # h-shifted upper-half copy, on-chip (pad cols never read)
            nc.vector.tensor_copy(X[64:128, 0:RPC, 1:W + 1, :],
                                  X[0:64, 1:RPC + 1, 1:W + 1, :])
            # fp8 image of slab rows 2..5 for the ki=2 "single" matmuls,
            # converted on-chip (split so the first rows unblock early)
            X8 = xpool.tile([64, RPC, W + 2, B], f8)
            nc.vector.tensor_copy(X8[:, 0:2, 1:W + 1, :],
                                  X[0:64, 2:4, 1:W + 1, :])
            nc.vector.tensor_copy(X8[:, 2:4, 1:W + 1, :],
                                  X[0:64, 4:6, 1:W + 1, :])

            bias_t = bpool.tile([128, LOCS], f16)
            nc.scalar.dma_start(bias_t, bias_d)

            out_rows = {}
            groups = _PLAN if ngroups is None else _PLAN[:ngroups]
            for rep in range(repeat):
              off = 0
              for gi, (s, n, blks) in enumerate(groups):
                  nb = len(blks)
                  bidx = {lk: i for i, lk in enumerate(blks)}
                  wp = wpool.tile([128, nb, O], f16, tag="wp",
                                  name=f"wp{rep}_{gi}")
                  ws = spool.tile([64, nb, O], f8, tag="ws",
                                  name=f"ws{rep}_{gi}")
                  wp_src = wp_d[:, off * O:(off + nb) * O] \
                      .rearrange("p (n o) -> p n o", o=O)
                  ws_src = ws_d[:, off * O:(off + nb) * O] \
                      .rearrange("p (n o) -> p n o", o=O)
                  nc.sync.dma_start(wp, wp_src)
                  (nc.scalar if mix == 0 else nc.sync).dma_start(ws, ws_src)
                  off += nb

                  for loc in range(s, s + n):
                      hol, wo = divmod(loc, WO)
                      if wo == 0:
                          out_rows[hol] = opool.tile([128, WO, B], f16,
                                                     tag="orow",
                                                     name=f"orow{rep}_{hol}")
                      orow = out_rows[hol]

                      last_row = hol == RPC - 1
                      if last_row:
                          if wo in _SEG_START:
                              seg_w = _SEG_START[wo]
                              seg0 = wo
                      elif wo % 8 == 0:
                          seg_w, seg0 = 8, wo
                      if wo == seg0:
                          ps = pspool.tile([128, seg_w, B], f32,
                                           tag=f"ps{seg_w}", bufs=4 if seg_w == 8 else 2,
                                           name=f"ps{rep}_{loc}")
                      kjs = [kj for kj in range(3) if not _skip(wo, kj)]
                      half = ps[:, wo - seg0, :]
                      for i, kj in enumerate(kjs):
                          nc.tensor.matmul(half, wp[:, bidx[(loc, kj)], :],
                                           X[:, hol, wo + kj, :],
                                           start=(i == 0), stop=False)
                      for i, kj in enumerate(kjs):
                          nc.tensor.matmul(half, ws[:, bidx[(loc, kj)], :],
                                           X8[:, hol, wo + kj, :],
                                           start=False,
                                           stop=(i == len(kjs) - 1))
                      seg_end = (wo - seg0 == seg_w - 1)
                      if seg_end:
                          nc.vector.tensor_tensor(
                              orow[:, seg0:wo + 1, :], ps,
                              bias_t[:, loc - (seg_w - 1):loc + 1, None]
                              .to_broadcast((128, seg_w, B)),
                              mybir.AluOpType.add)
                      # output DMAs ride the idle gpsimd (SWDGE) queue so
                      # their compute-dependent waits never block the weight
                      # streams on sync/scalar; the final chunks spread over
                      # scalar/sync (both idle by then) so the Pool desc-gen
                      # FIFO doesn't serialize the tail.
                      if last_row:
                          cw = _OUT_CHUNK.get(wo)
                          if cw is not None:
                              eng = {27: nc.scalar, 29: nc.scalar,
                                     31: nc.sync}.get(wo, nc.gpsimd)
                              eng.dma_start(
                                  out_d[hol, :, wo - cw + 1:wo + 1, :],
                                  orow[:, wo - cw + 1:wo + 1, :])
                      elif wo == WO - 1:
                          nc.gpsimd.dma_start(out_d[hol], orow)
    nc.finalize()
    return nc


def _prep_inputs(input, weight, bias):
    import ml_dtypes
    f8 = ml_dtypes.float8_e4m3
    inp = np.ascontiguousarray(input, dtype=np.float32)
    wgt = np.ascontiguousarray(weight, dtype=np.float32)
    bis = np.ascontiguousarray(bias, dtype=np.float32)

    in2 = np.ascontiguousarray(inp.transpose(2, 3, 1, 0))        # [h,w,c,b]
    # [ho,wo,kj,(ki01,c)=128,o] and [ho,wo,kj,c,o]
    wp_full = wgt[:, :, :, :, 0:2, :].transpose(0, 1, 5, 4, 3, 2) \
        .reshape(HO, WO, 3, 128, O).astype(np.float16)
    ws_full = wgt[:, :, :, :, 2, :].transpose(0, 1, 4, 3, 2).astype(f8)

    in_maps = []
    for core in range(NCORES):
        h0 = core * RPC
        # slab image without pad columns: [c=64, h', w, b]
        img = np.zeros((64, RPC + 2, W, B), np.float16)
        for hp in range(RPC + 2):
            h = h0 - 1 + hp
            if 0 <= h < H:
                img[:, hp] = in2[h].transpose(1, 0, 2)
        wp_blocks = []
        ws_blocks = []
        for s, n, blks in _PLAN:
            for loc, kj in blks:
                hol, wo = divmod(loc, WO)
                wp_blocks.append(wp_full[h0 + hol, wo, kj])   # [128, O]
                ws_blocks.append(ws_full[h0 + hol, wo, kj])   # [64, O]
        wp_c = np.stack(wp_blocks)          # [NBLK, 128, O]
        ws_c = np.stack(ws_blocks)          # [NBLK, 64, O]
        in_maps.append({
            "slab": img,
            "wp": np.ascontiguousarray(wp_c.transpose(1, 0, 2))
                .reshape(128, NBLK * O),
            "ws": np.ascontiguousarray(ws_c.transpose(1, 0, 2))
                .reshape(64, NBLK * O),
            "bias": np.ascontiguousarray(
                bis.reshape(O, HO, WO)[:, h0:h0 + RPC, :].reshape(O, LOCS))
                .astype(np.float16),
        })
    return in_maps


_RUN_KW = {}  # test.py can inject trace=True etc.
_LAST_RESULT = [None]
_NC_CACHE = [None]


def kernel(input, weight, bias):
    from concourse.bass_utils import run_bass_kernel_spmd

    in_maps = _prep_inputs(input, weight, bias)
    if _NC_CACHE[0] is None:
        _NC_CACHE[0] = _build_bass()
    nc = _NC_CACHE[0]
    res = run_bass_kernel_spmd(nc, in_maps, core_ids=list(range(NCORES)),
                               **_RUN_KW)
    _LAST_RESULT[0] = res
    arr = np.stack([r["out"] for r in res.results])   # [core,hol,o,wo,b]
    out = arr.astype(np.float32).transpose(4, 2, 0, 1, 3).reshape(B, O, HO, WO)
    return np.ascontiguousarray(out)


# revision 41
# speedup vs baseline: 1.6979x; 1.3569x over previous
"""Local2d (unshared-weight conv) Bass kernel for 8 trn2 NeuronCores.

Problem: input (64,64,32,32), weight (32,32,128,64,3,3), bias (128,32,32)
-> out (64,128,32,32).  K=3, stride 1, pad 1.

Sharding: spatial over h_out — core i handles output rows 4i..4i+3 and
reads the disjoint weight slice for those rows, plus a 6-row input halo
slab.

The kernel is DMA-bound (all traffic serializes through the shared DMA
engines at ~360B/ns), so the wire format is low precision against the
2e-2 tolerance (measured rel err of this exact scheme: 1.28e-2):
  - ALL weights and the input image travel as fp8 e3m4 (4 mantissa
    bits), scaled x32 / x2 on host to clear the e3m4 subnormal floor
    (min normal 0.25, max 15.5; zero saturation on this data);
  - the x64 product scale is divided back out in the PSUM merge;
  - bias and output travel as fp16 (PSUM accumulates fp32, host casts
    the output back to fp32).

DMA-byte trims:
  - matmuls that would touch an all-zero pad column (wo==0/kj==0,
    wo==31/kj==2) are skipped and their weight blocks never shipped;
  - pad columns of the input image are never transferred;
  - the weight stream is cut into groups with a tapered tail (8,...,8,
    4,2,1,1 locations), all weight-group buffers are resident (bufs=19)
    so the stream never stalls, and the output chunks ride the idle
    gpsimd/scalar queues so only a tiny compute chain trails the last
    weight byte.

Per output location (ho,wo), per valid kj, accumulating into one PSUM
group: one "paired" matmul, K=128 (partitions 0-63 = channels at ki=0,
64-127 = channels at ki=2; the image's upper half is the slab shifted
by two rows) and one "single" matmul, K=64 (channels at ki=1, read from
the image's lower half at row hol+1).  Merge per segment, split across
the idle engines: orow = ps8 * (1/64) on ACT, then += bias on DVE.
Stationary operand = per-location weights [K,128(o)], moving = input
columns [K,64(b)].  Host pre-transposes the weights so the contraction
dim lands on partitions with fully contiguous DMA.
"""

import numpy as np

B, C, O, KK, H, W = 64, 64, 128, 3, 32, 32
HO = WO = 32
NCORES = 8
RPC = HO // NCORES          # output rows per core
LOCS = RPC * WO             # locations per core

WSCALE = 32.0               # e3m4 weight scale (max |w|*32 ~ 7.2 < 15.5)
XSCALE = 2.0                # e3m4 input scale  (max |x|*2 ~ 10.2 < 15.5)

# location groups: one weight DMA (wq+ws) per group, tapered at the end
_GROUP_SIZES = [8] * 15 + [4, 2, 1, 1]


def _skip(wo, kj):
    return (wo == 0 and kj == 0) or (wo == WO - 1 and kj == 2)


def _group_plan():
    """[(loc_start, nlocs, [(loc, kj), ...]), ...] with block order shared
    by the host weight packer and the kernel builder."""
    plan, s = [], 0
    for n in _GROUP_SIZES:
        blks = []
        for loc in range(s, s + n):
            wo = loc % WO
            for kj in range(3):
                if not _skip(wo, kj):
                    blks.append((loc, kj))
        plan.append((s, n, blks))
        s += n
    assert s == LOCS
    return plan


_PLAN = _group_plan()
NBLK = sum(len(b) for _, _, b in _PLAN)

# last row: PSUM/merge segment starts -> width (rows 0-2 use 8 wide)
_SEG_START = {0: 8, 8: 8, 16: 8, 24: 4, 28: 2, 30: 2}
# output DMA chunks for the last row: wo -> chunk width
_OUT_CHUNK = {15: 16, 23: 8, 27: 4, 31: 4}


def _build_bass(mode="full", ngroups=None, mix=0, repeat=1):
    from concourse import bacc
    import concourse.mybir as mybir
    from concourse.tile import TileContext

    f16 = mybir.dt.float16
    f32 = mybir.dt.float32
    f8 = mybir.dt.float8e3
    nc = bacc.Bacc("TRN2", target_bir_lowering=False, debug=False,
                   num_devices=NCORES)

    # fp8 e3m4 input image (x2), dup baked on host, no pad columns:
    # full-width part: rows idx 0..3 (lower = slab rows 0..3, upper =
    # slab rows 2..5); x8b = lower-only slab row 4 (the upper half of
    # index 4 is never read, so its bytes are never shipped).
    x8_d = nc.dram_tensor("x8", (128, RPC, W, B), f8,
                          kind="ExternalInput").ap()
    x8b_d = nc.dram_tensor("x8b", (64, W, B), f8,
                           kind="ExternalInput").ap()
    # weights pre-arranged on host as one partition-major flat stream;
    # per-group slices are fully contiguous per partition.
    wq_d = nc.dram_tensor("wq", (128, NBLK * O), f8,
                          kind="ExternalInput").ap()
    ws_d = nc.dram_tensor("ws", (64, NBLK * O), f8,
                          kind="ExternalInput").ap()
    bias_d = nc.dram_tensor("bias", (O, LOCS), f16,
                            kind="ExternalInput").ap()
    out_d = nc.dram_tensor("out", (RPC, O, WO, B), f16,
                           kind="ExternalOutput").ap()

    with TileContext(nc) as tc:
        with tc.tile_pool(name="xslab", bufs=1) as xpool, \
             tc.tile_pool(name="wpool", bufs=19) as wpool, \
             tc.tile_pool(name="spool", bufs=19) as spool, \
             tc.tile_pool(name="opool", bufs=4) as opool, \
             tc.tile_pool(name="bpool", bufs=1) as bpool, \
             tc.tile_pool(name="psum", bufs=1, space="PSUM") as pspool:

            X8 = xpool.tile([128, RPC + 1, W + 2, B], f8)
            nc.sync.dma_start(X8[:, 0:RPC, 1:W + 1, :], x8_d)
            nc.sync.dma_start(X8[0:64, RPC, 1:W + 1, :], x8b_d)
            bias_t = bpool.tile([128, LOCS], f16)
            nc.scalar.dma_start(bias_t, bias_d)

            out_rows = {}
            groups = _PLAN if ngroups is None else _PLAN[:ngroups]
            for rep in range(repeat):
              off = 0
              for gi, (s, n, blks) in enumerate(groups):
                  nb = len(blks)
                  bidx = {lk: i for i, lk in enumerate(blks)}
                  wq = wpool.tile([128, nb, O], f8, tag="wq",
                                  name=f"wq{rep}_{gi}")
                  ws = spool.tile([64, nb, O], f8, tag="ws",
                                  name=f"ws{rep}_{gi}")
                  wq_src = wq_d[:, off * O:(off + nb) * O] \
                      .rearrange("p (n o) -> p n o", o=O)
                  ws_src = ws_d[:, off * O:(off + nb) * O] \
                      .rearrange("p (n o) -> p n o", o=O)
                  nc.sync.dma_start(wq, wq_src)
                  (nc.scalar if mix == 0 else nc.sync).dma_start(ws, ws_src)
                  off += nb

                  for loc in range(s, s + n):
                      hol, wo = divmod(loc, WO)
                      if wo == 0:
                          out_rows[hol] = opool.tile([128, WO, B], f16,
                                                     tag="orow",
                                                     name=f"orow{rep}_{hol}")
                      orow = out_rows[hol]

                      last_row = hol == RPC - 1
                      if last_row:
                          if wo in _SEG_START:
                              seg_w = _SEG_START[wo]
                              seg0 = wo
                      elif wo % 8 == 0:
                          seg_w, seg0 = 8, wo
                      if wo == seg0:
                          ps8 = pspool.tile([128, seg_w, B], f32,
                                            tag="ps8", bufs=8,
                                            name=f"ps8_{rep}_{loc}")
                      kjs = [kj for kj in range(3) if not _skip(wo, kj)]
                      h8 = ps8[:, wo - seg0, :]
                      # ki=1 single reads slab row hol+1 = X8 lower index
                      # hol+1 (the lower half carries rows 0..4)
                      xs = X8[0:64, hol + 1]
                      for i, kj in enumerate(kjs):
                          nc.tensor.matmul(h8, wq[:, bidx[(loc, kj)], :],
                                           X8[:, hol, wo + kj, :],
                                           start=(i == 0), stop=False)
                      for i, kj in enumerate(kjs):
                          nc.tensor.matmul(h8, ws[:, bidx[(loc, kj)], :],
                                           xs[:, wo + kj, :],
                                           start=False,
                                           stop=(i == len(kjs) - 1))
                      if wo - seg0 == seg_w - 1:
                          # orow = ps8/(WSCALE*XSCALE) + bias, split over
                          # the idle ACT engine (descale) and DVE (add) —
                          # walrus rejects the fused scalar_tensor_tensor
                          osl = orow[:, seg0:wo + 1, :]
                          mul_eng = nc.vector if (last_row and seg0 >= 24) \
                              else nc.scalar
                          if mul_eng is nc.scalar:
                              mul_eng.mul(osl, ps8, 1.0 / (WSCALE * XSCALE))
                          else:
                              mul_eng.tensor_scalar_mul(
                                  osl, ps8, 1.0 / (WSCALE * XSCALE))
                          nc.vector.tensor_tensor(
                              osl, osl,
                              bias_t[:, loc - (seg_w - 1):loc + 1, None]
                              .to_broadcast((128, seg_w, B)),
                              mybir.AluOpType.add)
                      # output DMAs ride the idle gpsimd (SWDGE) queue so
                      # their compute-dependent waits never block the weight
                      # streams on sync/scalar; the final chunks spread over
                      # scalar/sync (both idle by then).
                      if last_row:
                          cw = _OUT_CHUNK.get(wo)
                          if cw is not None:
                              eng = {27: nc.scalar,
                                     31: nc.sync}.get(wo, nc.gpsimd)
                              eng.dma_start(
                                  out_d[hol, :, wo - cw + 1:wo + 1, :],
                                  orow[:, wo - cw + 1:wo + 1, :])
                      elif wo == WO - 1:
                          nc.gpsimd.dma_start(out_d[hol], orow)
    nc.finalize()
    return nc


def _prep_inputs(input, weight, bias):
    import ml_dtypes
    f8x = ml_dtypes.float8_e3m4
    inp = np.ascontiguousarray(input, dtype=np.float32)
    wgt = np.ascontiguousarray(weight, dtype=np.float32)
    bis = np.ascontiguousarray(bias, dtype=np.float32)

    in2 = np.ascontiguousarray(inp.transpose(2, 3, 1, 0))        # [h,w,c,b]
    # paired fp8 blocks [ho,wo,kj,(ki0 c; ki2 c)=128,o], scaled x32
    wq_full = np.clip(
        wgt[:, :, :, :, (0, 2), :] * WSCALE, -15.5, 15.5) \
        .transpose(0, 1, 5, 4, 3, 2).reshape(HO, WO, 3, 128, O).astype(f8x)
    # fp8 single blocks [ho,wo,kj,c,o] (ki=1), same x32 scale
    ws_full = np.clip(wgt[:, :, :, :, 1, :] * WSCALE, -15.5, 15.5) \
        .transpose(0, 1, 4, 3, 2).astype(f8x)

    in_maps = []
    for core in range(NCORES):
        h0 = core * RPC
        # fp8 image: [128, 5, w, b]; lower = slab rows 0..4 x2 in e3m4,
        # upper = slab rows 2..5 (row index r holds slab row r+2)
        simg = np.zeros((64, RPC + 3, W, B), np.float32)
        for hp in range(RPC + 2):
            h = h0 - 1 + hp
            if 0 <= h < H:
                simg[:, hp] = in2[h].transpose(1, 0, 2)
        simg = simg.astype(np.float16).astype(np.float32) * XSCALE
        img = np.zeros((128, RPC, W, B), np.float32)
        img[0:64] = simg[:, 0:RPC]
        img[64:128] = simg[:, 2:RPC + 2]
        img = img.astype(f8x)
        imgb = simg[:, RPC].astype(f8x)
        wq_blocks = []
        ws_blocks = []
        for s, n, blks in _PLAN:
            for loc, kj in blks:
                hol, wo = divmod(loc, WO)
                wq_blocks.append(wq_full[h0 + hol, wo, kj])   # [128, O]
                ws_blocks.append(ws_full[h0 + hol, wo, kj])   # [64, O]
        wq_c = np.stack(wq_blocks)          # [NBLK, 128, O]
        ws_c = np.stack(ws_blocks)          # [NBLK, 64, O]
        in_maps.append({
            "x8": img,
            "x8b": imgb,
            "wq": np.ascontiguousarray(wq_c.transpose(1, 0, 2))
                .reshape(128, NBLK * O),
            "ws": np.ascontiguousarray(ws_c.transpose(1, 0, 2))
                .reshape(64, NBLK * O),
            "bias": np.ascontiguousarray(
                bis.reshape(O, HO, WO)[:, h0:h0 + RPC, :].reshape(O, LOCS))
                .astype(np.float16),
        })
    return in_maps


_RUN_KW = {}  # test.py can inject trace=True etc.
_LAST_RESULT = [None]
_NC_CACHE = [None]


def kernel(input, weight, bias):
    from concourse.bass_utils import run_bass_kernel_spmd

    in_maps = _prep_inputs(input, weight, bias)
    if _NC_CACHE[0] is None:
        _NC_CACHE[0] = _build_bass()
    nc = _NC_CACHE[0]
    res = run_bass_kernel_spmd(nc, in_maps, core_ids=list(range(NCORES)),
                               **_RUN_KW)
    _LAST_RESULT[0] = res
    arr = np.stack([r["out"] for r in res.results])   # [core,hol,o,wo,b]
    out = arr.astype(np.float32).transpose(4, 2, 0, 1, 3).reshape(B, O, HO, WO)
    return np.ascontiguousarray(out)
